# revision 61
# baseline (speedup 1.0000x reference)
"""GCN encoder (2-layer, PyG GCNConv w/ self-loops + symmetric norm) on 8 trn2 cores.

Math per layer: out = dis * ((A+I)(dis*x)) @ W + b, with dis = deg^-1/2, which
factorizes the per-edge norm dis[s]*dis[d] into a source row pre-scale and a
destination row post-scale.

Host side: x' = bf16(dis*x) is computed on host (input transform, like the
index tables), so the device does no prep pass. Destinations are permuted
into degree-balanced 128-row blocks (host un-permutes the output).

Device pipeline per core (destinations row-sharded, 49 blocks of 128 rows):
  layer 1: dma_gather 256B source rows per edge from x' (block-grouped,
           lo/hi split at row 32768 for int16 idx range)
           -> PE segment-sum transposed: stat=msg chunk, mov=one-hot S
              (built by 2x-mode is_equal) accumulating aggT[d, j] in PSUM
           -> epilogue (feature-major): z = W1^T aggT; transpose;
              x2 = dis*relu(dis*z + b1) via one DVE STT + one Act relu-scale
           -> x2own chunk write (partition-major layout, big DMA descs)
  exchange: K=4 chunked AllGathers of x2own (bf16), pipelined against L1
           production and L2 consumption (COLLECTIVE_CORES runs serially)
  layer 2: per AG chunk k: gathers from x2chunk_k; PSUM per (block, region),
           merged into an f32 SBUF partial; final epilogue per block:
           out = dis*(W2^T agg)^T + b2, written partition-major (host
           un-permutes).
"""

import sys

sys.path.insert(0, "/opt/trn_rl_repo")

import numpy as np
import ml_dtypes

BF16 = ml_dtypes.bfloat16

D = 128
P = 8
NCHUNK = 4
LO_SPLIT = 32768  # layer-1 source row split for int16 gather indices
FP8X = False  # fp8 exchange corrupts data through the runtime AllGather; keep bf16
L2_DELAY_MS = 0.16


def _sizes(n):
    rpc = -(-n // (P * 128)) * 128  # rows per core, multiple of 128
    npad = rpc * P
    b = rpc // 128  # dest blocks per core
    gs = 1
    for d_ in range(1, 9):
        if b % d_ == 0:
            gs = d_
    g = b // gs
    # AG chunk boundaries in blocks: small first chunk (starts the serial
    # collective chain early) and small last chunk (short tail before the
    # final L2 region), big middle chunks.
    if b == 49:
        bks = [4, 12, 26, 7]
    else:
        base = b // NCHUNK
        rem = b - base * NCHUNK
        bks = [base + (1 if k < rem else 0) for k in range(NCHUNK)]
    cb = []
    b0 = 0
    for bk in bks:
        cb.append((b0, b0 + bk))
        b0 += bk
    return rpc, npad, b, gs, g, cb


def plan(edge_index, n):
    """Host-side preprocessing: degree-balanced dest permutation, slot tables.

    Layer 1 gathers from x' (original node order), slots split lo/hi at row
    LO_SPLIT. Layer 2 gathers from the AG chunk tensors x2chunk_k; a source
    node's chunk is determined by its permuted block position.
    """
    rpc, npad, b, gs, g, cb = _sizes(n)
    nblocks = P * b
    src = edge_index[0].astype(np.int64)
    dst = edge_index[1].astype(np.int64)
    loops = np.arange(n, dtype=np.int64)
    allsrc = np.concatenate([src, loops])
    alldst = np.concatenate([dst, loops])

    deg = np.bincount(alldst, minlength=n).astype(np.float32)
    dis = 1.0 / np.sqrt(np.maximum(deg, 1.0))
    outdeg = np.bincount(src, minlength=n).astype(np.float64)

    # Destination permutation: in-degree balanced level-by-level (level l =
    # the l-th in-degree rank round), with OUT-degree steered within each
    # level: the last AG chunk's blocks get the lowest-out-degree nodes (its
    # L2 gather region is the serial tail), the first chunk next (head), and
    # the big mid-chunk with the largest AG window absorbs the highest.
    by_deg = np.argsort(-deg, kind="stable")
    steer = [3, 0, 2, 1] if len(cb) == 4 else list(range(len(cb)))
    bid_order = np.array(
        [
            c * b + blk
            for k in steer
            for c in range(P)
            for blk in range(cb[k][0], cb[k][1])
        ],
        dtype=np.int64,
    )
    perm_core = np.empty(n, dtype=np.int64)
    perm_blk = np.empty(n, dtype=np.int64)
    perm_slot = np.empty(n, dtype=np.int64)
    pos = 0
    lvl = 0
    while pos < n:
        cnt = min(nblocks, n - pos)
        nodes = by_deg[pos : pos + cnt]
        if cnt == nblocks:
            nodes = nodes[np.argsort(outdeg[nodes], kind="stable")]
            ab = bid_order
        else:
            ab = np.arange(cnt, dtype=np.int64)
        perm_core[nodes] = ab[:cnt] // b
        perm_blk[nodes] = ab[:cnt] % b
        perm_slot[nodes] = lvl
        pos += cnt
        lvl += 1

    # dis in permuted partition-major layout: dis_pm[core][p, b]
    dis_pm = np.zeros((P, 128, b), dtype=np.float32)
    dis_pm[perm_core, perm_slot, perm_blk] = dis

    dst_core = perm_core[alldst]
    dst_blk = perm_blk[alldst]
    drel = perm_slot[alldst].astype(np.float32)

    # layer-1 source rows: original order; lo/hi split
    hi1 = (allsrc >= LO_SPLIT).astype(np.int64)
    sidx1 = np.where(hi1 == 0, allsrc, allsrc - LO_SPLIT)

    # layer-2 source rows: x2chunk_k flat index
    # node (c, blk, p) in chunk k (cb[k][0] <= blk < cb[k][1]) at
    # flat row c*(128*bk) + p*bk + (blk - b0k)
    s_core = perm_core[allsrc]
    s_blk = perm_blk[allsrc]
    s_slot = perm_slot[allsrc]
    chunk_of_blk = np.zeros(b, dtype=np.int64)
    for k, (b0, b1) in enumerate(cb):
        chunk_of_blk[b0:b1] = k
    reg2 = chunk_of_blk[s_blk]
    bks = np.array([b1 - b0 for (b0, b1) in cb], dtype=np.int64)
    b0s = np.array([b0 for (b0, b1) in cb], dtype=np.int64)
    sidx2 = s_core * (128 * bks[reg2]) + s_slot * bks[reg2] + (s_blk - b0s[reg2])
    assert sidx1.max() < 32768 and sidx2.max() < 32768

    def slotize(sidx, cls, ncls, nedges=None):
        """Pack edges into per-(core, destblock, class) slot chunks.

        Returns per-core idx tables (one per class, block-grouped) and the
        drel table covering all classes' chunks consecutively. nedges
        restricts to the first nedges entries (drops self-loops for L2).
        """
        dc, db, dr_, sx, cl = dst_core, dst_blk, drel, sidx, cls
        if nedges is not None:
            dc, db, dr_ = dc[:nedges], db[:nedges], dr_[:nedges]
            sx, cl = sx[:nedges], cl[:nedges]
        key = (dc * b + db) * ncls + cl
        nkeys = P * b * ncls
        counts = np.bincount(key, minlength=nkeys)
        cc = counts.reshape(P, b, ncls)
        k2 = [max(1, int(-(-cc[:, :, j].max() // 128))) for j in range(ncls)]
        sp = [kk * 128 for kk in k2]
        order = np.argsort(key, kind="stable")
        key_s = key[order]
        run_start = np.zeros(nkeys, dtype=np.int64)
        np.cumsum(counts[:-1], out=run_start[1:])
        rank = np.arange(key_s.size, dtype=np.int64) - run_start[key_s]
        slot_base = np.zeros(nkeys + 1, dtype=np.int64)
        per_key = np.array([sp[j] for j in range(ncls)] * (P * b), dtype=np.int64)
        np.cumsum(per_key, out=slot_base[1:])
        pos = slot_base[key_s] + rank
        tot = int(slot_base[-1])
        idx_flat = np.zeros(tot, dtype=np.int16)
        drel_flat = np.full(tot, -1.0, dtype=np.float32)
        idx_flat[pos] = sx[order].astype(np.int16)
        drel_flat[pos] = dr_[order]
        spt = sum(sp)
        per_blk = idx_flat.reshape(P, b, spt)
        drel_blk = drel_flat.reshape(P, b, spt)
        k2t = sum(k2)

        def make_idx(core_slots, L):
            # [g, L] block-major slot lists -> [128, g*L/16] idx tile layout
            l16 = L // 16
            seq = core_slots.reshape(g, L)
            tile = seq.reshape(g, l16, 16).transpose(0, 2, 1)
            tile = np.tile(tile, (1, 8, 1))
            return np.ascontiguousarray(tile.transpose(1, 0, 2).reshape(128, g * l16))

        idx_cls = []  # per class: list per core of idx tables
        off = 0
        for j in range(ncls):
            L = gs * sp[j]
            idx_cls.append(
                [
                    make_idx(per_blk[c, :, off : off + sp[j]], L)
                    for c in range(P)
                ]
            )
            off += sp[j]
        drs = []
        for c in range(P):
            dr = drel_blk[c].reshape(b, k2t, 128).transpose(2, 0, 1)
            drs.append(np.ascontiguousarray(dr.reshape(128, b * k2t)).astype(BF16))
        return dict(k2=k2, k2t=k2t, idx=idx_cls, drel=drs)

    lay1 = slotize(sidx1, hi1, 2)
    # L2 drops self-loops (handled on-device via local transpose-accumulate)
    lay2 = slotize(sidx2, reg2, NCHUNK, nedges=src.size)
    k2tmax = max(lay1["k2t"], lay2["k2t"])

    per_core = []
    for c in range(P):
        per_core.append(
            {
                "idx1_lo": lay1["idx"][0][c],
                "idx1_hi": lay1["idx"][1][c],
                "drel1": lay1["drel"][c],
                "drel2": lay2["drel"][c],
                "dis_pm": np.ascontiguousarray(dis_pm[c]),
                **{f"idx2_{k}": lay2["idx"][k][c] for k in range(NCHUNK)},
            }
        )

    # iota_rep[p, j*k2tmax + c] = j  (chunk-minor for 2x-mode is_equal)
    iota_rep = np.repeat(np.arange(128, dtype=np.float32), k2tmax)
    iota_rep = np.tile(iota_rep, (128, 1)).astype(BF16)
    ident = np.eye(128, dtype=np.float32)
    return {
        "sizes": (rpc, npad, b, gs, g, tuple(cb), k2tmax,
                  tuple(lay1["k2"]), tuple(lay2["k2"])),
        "per_core": per_core,
        "perm": (perm_core, perm_blk, perm_slot),
        "dis": dis,
        "iota_rep": iota_rep,
        "ident_bf": ident.astype(BF16),
    }


def build_program(pl):
    import concourse.mybir as mybir
    from concourse.bacc import Bacc
    from concourse.tile import TileContext

    (rpc, npad, b, gs, g, cb, k2tmax, k2l1, k2l2) = pl["sizes"]
    f32 = mybir.dt.float32
    bf16 = mybir.dt.bfloat16
    i16 = mybir.dt.int16
    AF = mybir.ActivationFunctionType
    OP = mybir.AluOpType
    k2t1 = sum(k2l1)
    k2t2 = sum(k2l2)

    nc = Bacc(num_devices=P)

    xp_in = nc.declare_dram_parameter("xp", [npad, D], bf16, isOutput=False)
    dis_in = nc.declare_dram_parameter("dis_pm", [128, b], f32, isOutput=False)
    w1_in = nc.declare_dram_parameter("W1", [D, D], f32, isOutput=False)
    w2_in = nc.declare_dram_parameter("W2", [D, D], f32, isOutput=False)
    b1t_in = nc.declare_dram_parameter("b1_tile", [D, D], f32, isOutput=False)
    b2t_in = nc.declare_dram_parameter("b2_tile", [D, D], f32, isOutput=False)
    iota_in = nc.declare_dram_parameter(
        "iota_rep", [128, 128 * k2tmax], bf16, isOutput=False
    )
    identb_in = nc.declare_dram_parameter("ident_bf", [128, 128], bf16, isOutput=False)
    idx1lo_in = nc.declare_dram_parameter(
        "idx1_lo", [128, g * gs * k2l1[0] * 8], i16, isOutput=False
    )
    idx1hi_in = nc.declare_dram_parameter(
        "idx1_hi", [128, g * gs * k2l1[1] * 8], i16, isOutput=False
    )
    idx2_in = [
        nc.declare_dram_parameter(
            f"idx2_{k}", [128, g * gs * k2l2[k] * 8], i16, isOutput=False
        )
        for k in range(NCHUNK)
    ]
    drel1_in = nc.declare_dram_parameter("drel1", [128, b * k2t1], bf16, isOutput=False)
    drel2_in = nc.declare_dram_parameter("drel2", [128, b * k2t2], bf16, isOutput=False)
    out = nc.declare_dram_parameter("out", [128, b, D], f32, isOutput=True)

    # AG chunk tensors: in = [128, bk, 128] partition-major; out stacks cores
    fp8 = mybir.dt.float8e4
    xdt = fp8 if FP8X else bf16
    x2own = []
    x2chunk = []
    x2bf = []
    for k, (b0, b1) in enumerate(cb):
        bk = b1 - b0
        x2own.append(nc.dram_tensor(f"x2own_{k}", [128, bk, D], xdt))
        x2chunk.append(
            nc.dram_tensor(f"x2chunk_{k}", [P, 128, bk, D], xdt, addr_space="Shared")
        )
        if FP8X:
            x2bf.append(nc.dram_tensor(f"x2bf_{k}", [P, 128, bk, D], bf16))
        else:
            x2bf.append(x2chunk[k])

    with TileContext(nc) as tc:
        with (
            tc.tile_pool(name="const", bufs=1) as const,
            tc.tile_pool(name="msgs", bufs=5) as msgs,
            tc.tile_pool(name="spool", bufs=3) as spool,
            tc.tile_pool(name="yout", bufs=3) as yout,
            tc.tile_pool(name="epi", bufs=8) as epi,
            tc.tile_pool(name="upc", bufs=2) as upc,
            tc.tile_pool(name="fin", bufs=2) as fin,
            tc.tile_pool(name="pa", bufs=4, space="PSUM") as pa,
            tc.tile_pool(name="pt", bufs=2, space="PSUM") as pt,
            tc.tile_pool(name="pz", bufs=2, space="PSUM") as pz,
        ):
            # ---- constants -------------------------------------------------
            def load_const(param, shape, dtype, tag):
                t = const.tile(shape, dtype, tag=tag)
                nc.sync.dma_start(t[:], param[:])
                return t

            # L1-critical consts first: the first gather waits on its idx
            # table; everything L2-only is loaded after L1 is emitted.
            idx1lo_sb = load_const(
                idx1lo_in, [128, g * gs * k2l1[0] * 8], i16, "idx1lo"
            )
            idx1hi_sb = load_const(
                idx1hi_in, [128, g * gs * k2l1[1] * 8], i16, "idx1hi"
            )
            drel1_sb = load_const(drel1_in, [128, b * k2t1], bf16, "drel1")
            iota_sb = load_const(iota_in, [128, 128 * k2tmax], bf16, "iota")
            dis_sb = load_const(dis_in, [128, b], f32, "dis")
            w1_sb = load_const(w1_in, [D, D], f32, "w1")
            b1t_sb = load_const(b1t_in, [D, D], f32, "b1t")
            identb_sb = load_const(identb_in, [128, 128], bf16, "identb")
            # L2-final consts: keep their DMA off the critical early window
            with tc.tile_wait_until(0.25):
                w2_sb = load_const(w2_in, [D, D], f32, "w2")
                b2t_sb = load_const(b2t_in, [D, D], f32, "b2t")

            w1b = const.tile([D, D], bf16, tag="w1b")
            nc.vector.tensor_copy(w1b[:], w1_sb[:])
            w2b = const.tile([D, D], bf16, tag="w2b")
            nc.vector.tensor_copy(w2b[:], w2_sb[:])

            # f32 partial for layer-2 aggregation (aggT layout [d, j] per block)
            partial = const.tile([128, b * 128], f32, tag="partial")

            # msg tile: sized for the largest gather unit actually issued
            mcols = max(4 * k2l1[0], 4 * k2l1[1], gs * max(k2l2))

            def gather(idx_sb, src_ap, k2h, gg, j0=0, nblk=gs):
                # gather chunks for blocks [gg*gs+j0, gg*gs+j0+nblk)
                L = nblk * k2h * 128
                l16g = gs * k2h * 8  # idx cols per group
                col0 = gg * l16g + j0 * k2h * 8
                msg = msgs.tile([128, mcols, D], bf16, tag="msg")
                nc.gpsimd.dma_gather(
                    msg[:, 0 : nblk * k2h, :],
                    src_ap,
                    idx_sb[:, col0 : col0 + L // 16],
                    L,
                    L,
                    D,
                    single_packet=False,
                )
                return msg

            def build_S(drel_sb, k2t, bb, koff=0, nch=None):
                # one-hot S slice for chunks [koff, koff+nch) of block bb
                if nch is None:
                    nch = k2t
                S = spool.tile([128, 128, k2tmax], bf16, tag="S")
                nc.vector.tensor_tensor(
                    S[:, :, 0:nch],
                    iota_sb[:, :].rearrange("p (j c) -> p j c", j=128)[:, :, 0:nch],
                    drel_sb[:, bb * k2t + koff : bb * k2t + koff + nch]
                    .rearrange("p (a c) -> p a c", a=1)
                    .broadcast_to([128, 128, nch]),
                    OP.is_equal,
                )
                return S

            # ---- layer 1 ---------------------------------------------------
            # per group: gather lo+hi, per block: S, 19 matmuls -> aggT PSUM,
            # epilogue -> x2stage [j, hid] bf16, write per chunk
            k2lo, k2hi = k2l1

            def l1_unit(gg, j0, nblk):
                mlo = gather(idx1lo_sb, xp_in[0:LO_SPLIT, :], k2lo, gg, j0, nblk)
                mhi = gather(idx1hi_sb, xp_in[LO_SPLIT:npad, :], k2hi, gg, j0, nblk)
                ystage = yout.tile([128, gs, D], fp8 if FP8X else bf16, tag="yst")
                for j in range(nblk):
                    bb = gg * gs + j0 + j
                    S = build_S(drel1_sb, k2t1, bb)
                    aggT = pa.tile([128, D], f32, tag="aggT")
                    for c in range(k2lo):
                        nc.tensor.matmul(
                            aggT[:],
                            mlo[:, j * k2lo + c, :],
                            S[:, :, c],
                            start=(c == 0),
                            stop=False,
                        )
                    for c in range(k2hi):
                        nc.tensor.matmul(
                            aggT[:],
                            mhi[:, j * k2hi + c, :],
                            S[:, :, k2lo + c],
                            start=False,
                            stop=(c == k2hi - 1),
                        )
                    # z = W1^T aggT  [hid, j]
                    aggb = epi.tile([128, D], bf16, tag="aggb")
                    nc.scalar.activation(aggb[:], aggT[:], AF.Copy)
                    z_p = pz.tile([128, D], f32, tag="z_p")
                    nc.tensor.matmul(z_p[:], w1b[:], aggb[:], start=True, stop=True)
                    zs = epi.tile([128, D], bf16, tag="zs")
                    nc.scalar.activation(zs[:], z_p[:], AF.Copy)
                    zT_p = pt.tile([128, D], bf16, tag="zT_p")
                    nc.tensor.transpose(zT_p[:], zs[:], identb_sb[:])
                    # tmp = dis_j * zT + b1_tile ; x2 = relu(dis_j * tmp)
                    tmp = epi.tile([128, D], f32, tag="tmp")
                    nc.vector.scalar_tensor_tensor(
                        tmp[:],
                        zT_p[:],
                        dis_sb[:, bb : bb + 1],
                        b1t_sb[:],
                        OP.mult,
                        OP.add,
                    )
                    nc.scalar.activation(
                        ystage[:, j, :],
                        tmp[:],
                        AF.Relu,
                        scale=dis_sb[:, bb : bb + 1],
                    )
                # write ystage blocks into their chunks (partition-major);
                # high priority so the write preempts queued gathers on the
                # DMA engines (it gates the serial AllGather chain)
                b0u = gg * gs + j0
                ju = 0
                with tc.high_priority():
                    while ju < nblk:
                        blk = b0u + ju
                        k = next(
                            i for i, (c0, c1) in enumerate(cb) if c0 <= blk < c1
                        )
                        c0, c1 = cb[k]
                        take = min(nblk - ju, c1 - blk)
                        nc.sync.dma_start(
                            x2own[k][:, blk - c0 : blk - c0 + take, :],
                            ystage[:, ju : ju + take, :],
                        )
                        ju += take

            # ---- layer 2 region pass --------------------------------------
            def load_own_group(gg):
                # own block rows (self-loop sources), cast to bf16 for PE
                xgrp = yout.tile([128, gs, D], fp8 if FP8X else bf16, tag="xgrp")
                b0g = gg * gs
                j0 = 0
                while j0 < gs:
                    blk = b0g + j0
                    k = next(i for i, (c0, c1) in enumerate(cb) if c0 <= blk < c1)
                    c0, c1 = cb[k]
                    take = min(gs - j0, c1 - blk)
                    nc.sync.dma_start(
                        xgrp[:, j0 : j0 + take, :],
                        x2own[k][:, blk - c0 : blk - c0 + take, :],
                    )
                    j0 += take
                if not FP8X:
                    return xgrp
                xgb = yout.tile([128, gs, D], bf16, tag="xgb")
                nc.scalar.activation(xgb[:, :, :], xgrp[:, :, :], AF.Copy)
                return xgb

            def upconvert(k):
                # fp8 AG output -> bf16 gather source, per sender core slice
                bk = cb[k][1] - cb[k][0]
                for cc in range(P):
                    t8 = upc.tile([128, 20, D], fp8, tag="u8")
                    nc.sync.dma_start(t8[:, 0:bk, :], x2chunk[k][cc, :, :, :])
                    tb = upc.tile([128, 20, D], bf16, tag="ub")
                    nc.scalar.activation(tb[:, 0:bk, :], t8[:, 0:bk, :], AF.Copy)
                    nc.sync.dma_start(x2bf[k][cc, :, :, :], tb[:, 0:bk, :])

            def l2_region(k, gg, first):
                k2r = k2l2[k]
                koff = sum(k2l2[:k])
                src = x2bf[k][:, :, :, :].rearrange("c p b d -> (c p b) d")
                m = gather(idx2_sb[k], src, k2r, gg)
                xgrp = load_own_group(gg) if first else None
                last = k == NCHUNK - 1
                for j in range(gs):
                    bb = gg * gs + j
                    if last:
                        S = s3all[:, :, bb * k2r : (bb + 1) * k2r]
                    else:
                        S = build_S(drel2_sb, k2t2, bb, koff=koff, nch=k2r)
                    aggT = pa.tile([128, D], f32, tag="aggT")
                    if first:
                        # self-loop contribution: aggT[d, j'] += xgrp[j', d]
                        # (stat=xgrp, mov=identity == transpose, f32 accum)
                        nc.tensor.matmul(
                            aggT[:],
                            xgrp[:, j, :],
                            identb_sb[:],
                            start=True,
                            stop=False,
                        )
                    for c in range(k2r):
                        nc.tensor.matmul(
                            aggT[:],
                            m[:, j * k2r + c, :],
                            S[:, :, c],
                            start=(not first) and (c == 0),
                            stop=(c == k2r - 1),
                        )
                    if first:
                        nc.scalar.activation(
                            partial[:, bb * 128 : (bb + 1) * 128], aggT[:], AF.Copy
                        )
                    elif k < NCHUNK - 1:
                        nc.vector.scalar_tensor_tensor(
                            partial[:, bb * 128 : (bb + 1) * 128],
                            aggT[:],
                            1.0,
                            partial[:, bb * 128 : (bb + 1) * 128],
                            OP.mult,
                            OP.add,
                        )
                    else:
                        # last region: merge straight into the bf16 matmul
                        # input (saves a partial round-trip in the tail)
                        aggb = fin.tile([128, D], bf16, tag=f"aggf{j}")
                        nc.vector.scalar_tensor_tensor(
                            aggb[:],
                            aggT[:],
                            1.0,
                            partial[:, bb * 128 : (bb + 1) * 128],
                            OP.mult,
                            OP.add,
                        )
                        aggfinal[j] = aggb

            def l2_final(gg):
                ostage = yout.tile([128, gs, D], f32, tag="ost")
                for j in range(gs):
                    bb = gg * gs + j
                    aggb = aggfinal[j]
                    z_p = pz.tile([128, D], f32, tag="z_p")
                    nc.tensor.matmul(z_p[:], w2b[:], aggb[:], start=True, stop=True)
                    zs = epi.tile([128, D], bf16, tag="zs")
                    nc.scalar.activation(zs[:], z_p[:], AF.Copy)
                    zT_p = pt.tile([128, D], bf16, tag="zT_p")
                    nc.tensor.transpose(zT_p[:], zs[:], identb_sb[:])
                    nc.vector.scalar_tensor_tensor(
                        ostage[:, j, :],
                        zT_p[:],
                        dis_sb[:, bb : bb + 1],
                        b2t_sb[:],
                        OP.mult,
                        OP.add,
                    )
                nc.sync.dma_start(
                    out[:, gg * gs : (gg + 1) * gs, :], ostage[:, :, :]
                )

            # ---- schedule: L1 units, AGs as chunks complete, L2 regions --
            done_chunk = [False] * NCHUNK
            blocks_done = 0

            def group_units(gg):
                # unit boundaries: chunk edges (so AGs fire promptly) plus
                # cuts keeping units <= 4 blocks
                lo, hi = gg * gs, (gg + 1) * gs
                cuts = {lo, hi}
                for (c0, c1) in cb:
                    if lo < c1 < hi:
                        cuts.add(c1)
                pts = sorted(cuts)
                units = []
                for a, bnd in zip(pts, pts[1:]):
                    seg = bnd - a
                    while seg > 4:
                        half = (seg + 1) // 2 if seg <= 8 else 4
                        units.append(half)
                        seg -= half
                    units.append(seg)
                return units

            for gg in range(g):
                j0 = 0
                for nblk in group_units(gg):
                    l1_unit(gg, j0, nblk)
                    j0 += nblk
                    blocks_done += nblk
                    for k, (c0, c1) in enumerate(cb):
                        if not done_chunk[k] and blocks_done >= c1:
                            with tc.high_priority():
                                nc.gpsimd.collective_compute(
                                    "AllGather",
                                    mybir.AluOpType.bypass,
                                    replica_groups=[list(range(P))],
                                    ins=[x2own[k][:, :, :]],
                                    outs=[x2chunk[k][:, :, :, :]],
                                )
                            if FP8X:
                                with tc.tile_wait_until(L2_DELAY_MS - 0.01):
                                    upconvert(k)
                            done_chunk[k] = True
            # L2-only consts, staggered: each needed only when its AG lands
            idx2_sb = []
            for k in range(NCHUNK):
                with tc.tile_wait_until(0.035 + 0.08 * k):
                    idx2_sb.append(
                        load_const(
                            idx2_in[k], [128, g * gs * k2l2[k] * 8], i16, f"idx2_{k}"
                        )
                    )
            with tc.tile_wait_until(0.035):
                drel2_sb = load_const(drel2_in, [128, b * k2t2], bf16, "drel2")

            # prebuild ALL last-region S matrices in one early is_equal (the
            # tail otherwise pays 49 small DVE builds after the final AG)
            k2last = k2l2[NCHUNK - 1]
            kofflast = sum(k2l2[: NCHUNK - 1])
            s3all = const.tile([128, 128, b * k2last], bf16, tag="s3all")
            nc.vector.tensor_tensor(
                s3all[:, :, :].rearrange("p j (b c) -> p j b c", b=b),
                iota_sb[:, 0 : 128 * k2tmax]
                .rearrange("p (j x) -> p j x", j=128)[:, :, 0:k2last]
                .unsqueeze(2)
                .broadcast_to([128, 128, b, k2last]),
                drel2_sb[:, :]
                .rearrange("p (b x) -> p b x", b=b)[:, :, kofflast : kofflast + k2last]
                .unsqueeze(1)
                .broadcast_to([128, 128, b, k2last]),
                OP.is_equal,
            )
            aggfinal = [None] * gs
            for k in range(NCHUNK):
                # keep L2 region DMA out of layer 1's window: layer-1 gathers
                # pace the serial AllGather chain, so they get the DMA first
                with tc.tile_wait_until(L2_DELAY_MS, enable=FP8X):
                    for gg in range(g):
                        l2_region(k, gg, first=(k == 0))
                        if k == NCHUNK - 1:
                            l2_final(gg)

    nc.finalize()
    return nc


def make_in_maps(pl, x, w1, b1, w2, b2):
    n = x.shape[0]
    (rpc, npad, b, gs, g, cb, k2tmax, k2l1, k2l2) = pl["sizes"]
    dis = pl["dis"]
    xp = np.zeros((npad, D), dtype=BF16)
    xp[:n] = (x.astype(np.float32) * dis[:, None]).astype(BF16)
    shared = {
        "xp": xp,
        "W1": np.ascontiguousarray(w1.astype(np.float32)),
        "W2": np.ascontiguousarray(w2.astype(np.float32)),
        "b1_tile": np.ascontiguousarray(
            np.tile(b1.astype(np.float32).reshape(1, D), (D, 1))
        ),
        "b2_tile": np.ascontiguousarray(
            np.tile(b2.astype(np.float32).reshape(1, D), (D, 1))
        ),
        "iota_rep": pl["iota_rep"],
        "ident_bf": pl["ident_bf"],
    }
    in_maps = []
    for c in range(P):
        m = dict(shared)
        pc = pl["per_core"][c]
        m["dis_pm"] = pc["dis_pm"]
        m["idx1_lo"] = pc["idx1_lo"]
        m["idx1_hi"] = pc["idx1_hi"]
        m["drel1"] = pc["drel1"]
        m["drel2"] = pc["drel2"]
        for k in range(NCHUNK):
            m[f"idx2_{k}"] = pc[f"idx2_{k}"]
        in_maps.append(m)
    return in_maps


_CACHE = {}


def kernel(x, edge_index, W1, b1, W2, b2):
    from concourse.bass_utils import run_bass_kernel_spmd

    x = np.asarray(x)
    edge_index = np.asarray(edge_index)
    n = x.shape[0]
    pl = plan(edge_index, n)
    key = pl["sizes"]
    if key not in _CACHE:
        _CACHE[key] = build_program(pl)
    nc = _CACHE[key]
    in_maps = make_in_maps(
        pl, x, np.asarray(W1), np.asarray(b1), np.asarray(W2), np.asarray(b2)
    )
    last_err = None
    for backoff in (15.0, 45.0, 0.0):
        try:
            r = run_bass_kernel_spmd(nc, in_maps, list(range(P)))
            break
        except Exception as ex:  # transient NRT/axon failures wedge briefly
            last_err = ex
            if backoff:
                import time

                time.sleep(backoff)
    else:
        raise last_err

    perm_core, perm_blk, perm_slot = pl["perm"]
    outs = np.stack([r.results[c]["out"] for c in range(P)], axis=0)
    # outs[c][p, b, d] -> node rows
    res = outs[perm_core, perm_slot, perm_blk]
    return np.ascontiguousarray(res).astype(np.float32)
